# revision 1
# baseline (speedup 1.0000x reference)
"""FlowNet correlation kernel for Trainium2 (8 NeuronCores, batch-parallel).

Problem: out[b, d, y, x] = (1/C) * sum_c i1[b,c,y,x] * pad(i2)[b,c,y+dy,x+dx]
  B=8, C=256, H=48, W=64, pad=20, displacements dy,dx in {-20..20 step 2}
  (21x21 = 441), output [8, 441, 48, 64] fp32.

Strategy (per core, one batch element):
  Displacement stride 2 => the problem splits into 4 independent polyphase
  subproblems (y-parity sy, x-parity sx), each a dense +-10 correlation on a
  24x32 quarter image. For each subproblem and each block of 4 sub-rows
  (M = 4*32 = 128 output pixels), compute the all-pairs band via fp32
  matmuls: stationary = i1 block [C, 128], streaming = the padded-i2 window
  (24 sub-rows x 52 sub-cols = 1248 cols, split 468/468/312 to respect the
  512-fp32 PSUM bank limit), accumulating over the two 128-channel k-tiles.
  Scale by 1/C during the PSUM->SBUF copy, then extract the 441 per-pixel
  correlation values with diagonal-access-pattern DMAs (flat SBUF addressing
  couples partition and byte strides) writing directly to HBM in
  [y, x, d] layout (1764-byte contiguous runs). Host transposes to [d, y, x].
"""

import numpy as np

C = 256
H, W = 48, 64
ND = 21          # displacements per axis
D = ND * ND      # 441
SUB_H, SUB_W = H // 2, W // 2      # 24, 32
HP, WP = H + 40, W + 40            # padded full-res 88, 104
BAND_W = 52                        # padded sub-cols
BAND_ROWS = 24                     # window sub-rows per block
BAND_N = BAND_ROWS * BAND_W        # 1248
ROW_SPLITS = [(0, 9), (9, 18), (18, 24)]   # window-row ranges per PSUM bank
N_BLOCKS = SUB_H // 4              # 6

_CACHE = {}


def _build():
    import concourse.bacc as bacc
    import concourse.mybir as mybir
    from concourse.bass_types import AP, SBTensorHandle
    from concourse.tile import TileContext

    f32 = mybir.dt.float32

    def alias_sbuf(nc, name, shape, dtype, offset, base_partition):
        # SBUF tensor view at a fixed byte offset and nonzero base partition.
        # Mirrors alloc_sbuf_tensor_at but rebases the partition origin so
        # diagonal gather APs keep their flat offset inside one partition row
        # (walrus rejects partition-crossing offsets on irregular APs).
        uname = nc._get_name(name, add_next_id=True)
        nc._tensor(uname, list(shape), dtype, type="SB")
        import functools, operator
        per_part = functools.reduce(operator.mul, shape[1:]) * mybir.dt.size(dtype)
        h = SBTensorHandle(
            uname,
            list(shape),
            dtype,
            base_partition=base_partition,
            manual_sbuf_range=(offset, offset + per_part),
            manual_base_name=name,
        )
        mloc = nc.lookup_mloc(h)
        mloc.allocated = True
        mloc.addr = offset
        mloc.base = base_partition
        return h
    nc = bacc.Bacc("TRN2", target_bir_lowering=False, debug=False)
    i1_t = nc.dram_tensor("i1", [C, H, W], f32, kind="ExternalInput")
    i2_t = nc.dram_tensor("i2", [C, H, W], f32, kind="ExternalInput")
    od_t = nc.dram_tensor("od", [H, W, D], f32, kind="ExternalOutput")

    NBUF = 3
    band_full = []
    band_alias = []
    for i in range(NBUF):
        h = nc.alloc_sbuf_tensor(f"bandf{i}", [128, BAND_N], f32)
        addr = nc.lookup_mloc(h).addr
        band_full.append(h)
        band_alias.append(
            [
                alias_sbuf(nc, f"band{i}ry{ry}", [32, BAND_N], f32, addr, 32 * ry)
                for ry in range(4)
            ]
        )

    from bass_rust import add_dep_helper

    last_gathers = [[] for _ in range(NBUF)]

    with TileContext(nc) as tc:
        with (
            tc.tile_pool(name="inp", bufs=1) as inp_pool,
            tc.tile_pool(name="ps", bufs=2, space="PSUM") as ps_pool,
        ):
            i1_sb = [
                inp_pool.tile([128, H * W], f32, name=f"i1k{k}", tag=f"i1k{k}") for k in range(2)
            ]
            i2_sb = [
                inp_pool.tile([128, HP * WP], f32, name=f"i2k{k}", tag=f"i2k{k}") for k in range(2)
            ]
            i1s_sb = [
                [
                    inp_pool.tile(
                        [128, SUB_H * SUB_W], f32, name=f"i1s{k}{s}", tag=f"i1s{k}{s}"
                    )
                    for s in range(4)
                ]
                for k in range(2)
            ]
            i1v = [t[:].rearrange("c (h w) -> c h w", h=H) for t in i1_sb]
            i2v = [t[:].rearrange("c (h w) -> c h w", h=HP) for t in i2_sb]

            for k in range(2):
                cs = slice(128 * k, 128 * (k + 1))
                nc.sync.dma_start(out=i1_sb[k][:], in_=i1_t.ap()[cs])
                v = i2v[k]
                # zero the pad ring (gpsimd; disjoint from the interior DMA)
                nc.gpsimd.memset(v[:, 0:20, :], 0.0)
                nc.gpsimd.memset(v[:, 68:HP, :], 0.0)
                nc.gpsimd.memset(v[:, 20:68, 0:20], 0.0)
                nc.gpsimd.memset(v[:, 20:68, 84:WP], 0.0)
                nc.sync.dma_start(out=v[:, 20:68, 20:84], in_=i2_t.ap()[cs])
                # de-interleave i1 into the 4 polyphase sub-images (gpsimd):
                # stationary matmul operands need a single-stride free dim
                for s in range(4):
                    sy, sx = s >> 1, s & 1
                    nc.gpsimd.tensor_copy(
                        i1s_sb[k][s][:].rearrange(
                            "c (py px) -> c py px", py=SUB_H
                        ),
                        i1v[k][:, sy : sy + 2 * SUB_H - 1 : 2, sx::2],
                    )

            inv_c = 1.0 / C
            for s in range(4):
                sy, sx = s >> 1, s & 1
                for yb in range(N_BLOCKS):
                    Y = 4 * yb
                    ps = ps_pool.tile([128, 1536], f32, name="ps")
                    for j, (r0, r1) in enumerate(ROW_SPLITS):
                        n = (r1 - r0) * BAND_W
                        for k in range(2):
                            lhs = i1s_sb[k][s][:, 32 * Y : 32 * Y + 128]
                            rh = i2v[k][
                                :,
                                2 * (Y + r0) + sy : 2 * (Y + r1 - 1) + sy + 1 : 2,
                                sx::2,
                            ]
                            nc.tensor.matmul(
                                ps[:, 512 * j : 512 * j + n],
                                lhsT=lhs,
                                rhs=rh,
                                start=(k == 0),
                                stop=(k == 1),
                            )
                    bi = (s * N_BLOCKS + yb) % NBUF
                    band = band_full[bi].ap()
                    copies = [
                        nc.vector.tensor_scalar_mul(
                            band[:, 0:468], ps[:, 0:468], inv_c
                        ),
                        nc.vector.tensor_scalar_mul(
                            band[:, 468:936], ps[:, 512:980], inv_c
                        ),
                        nc.scalar.mul(band[:, 936:1248], ps[:, 1024:1336], inv_c),
                    ]
                    # band buffers live outside the tile pools (the gather
                    # aliases rebase partitions, which Tile can't track), so
                    # RAW (gather-after-copy) and WAR (copy-after-gather on
                    # buffer reuse) edges are added explicitly.
                    for c in copies:
                        for g in last_gathers[bi]:
                            add_dep_helper(c.ins, g.ins, reason="band WAR")
                    gathers = []
                    for ry in range(4):
                        rd = AP(
                            band_alias[bi][ry],
                            ry * BAND_W,
                            [[BAND_N + 1, 32], [BAND_W, ND], [1, ND]],
                        )
                        wr = AP(
                            od_t.ap().tensor,
                            (2 * (Y + ry) + sy) * (W * D) + sx * D,
                            [[2 * D, 32], [ND, ND], [1, ND]],
                        )
                        g = nc.sync.dma_start(out=wr, in_=rd)
                        for c in copies:
                            add_dep_helper(g.ins, c.ins, reason="band RAW")
                        gathers.append(g)
                    last_gathers[bi] = gathers

    nc.compile()
    return nc


def _get_program():
    if "nc" not in _CACHE:
        _CACHE["nc"] = _build()
    return _CACHE["nc"]


def kernel(input1: np.ndarray, input2: np.ndarray) -> np.ndarray:
    from concourse import bass_utils

    nc = _get_program()
    input1 = np.ascontiguousarray(input1, dtype=np.float32)
    input2 = np.ascontiguousarray(input2, dtype=np.float32)
    B = input1.shape[0]
    in_maps = [{"i1": input1[b], "i2": input2[b]} for b in range(B)]
    res = bass_utils.run_bass_kernel_spmd(nc, in_maps, core_ids=list(range(B)))
    out = np.stack([r["od"] for r in res.results])  # [B, H, W, D]
    return np.ascontiguousarray(out.transpose(0, 3, 1, 2))  # [B, D, H, W]



# revision 7
# speedup vs baseline: 1.9531x; 1.9531x over previous
"""FlowNet correlation kernel for Trainium2 (8 NeuronCores, batch-parallel).

Problem: out[b, d, y, x] = (1/C) * sum_c i1[b,c,y,x] * pad(i2)[b,c,y+dy,x+dx]
  B=8, C=256, H=48, W=64, pad=20, displacements dy,dx in {-20..20 step 2}
  (21x21 = 441), output [8, 441, 48, 64] fp32.

Strategy (per core, one batch element):
  Displacement stride 2 => 4 polyphase subproblems (sy, sx), each a dense
  +-10 correlation on a 24x32 quarter image. Per subproblem, blocks of 4
  sub-rows (M = 4*32 = 128 pixels): all-pairs band via bf16 matmuls
  (1 cycle/row, 4x fp32 rate; inputs convert during the on-chip
  restride/deinterleave copies), band = 24 win-rows x 52
  win-cols = 1248, accumulated over two 128-channel k-tiles.

  Extraction avoids tiny-element DMAs (the old 84B diagonal gathers ran at
  ~6ns/elem contended): PSUM -> bf16 band (vector, cast), then ONE
  per-partition diagonal *shift* DMA per ry-group (elements are whole
  1061-elem rows, 2.1KB), then a regular vector re-stride 52->21 with the
  1/C scale into a compact [pixel, 441] fp32 tile, then a fully regular
  contiguous write DMA to HBM ([y, x, d] layout, 1764B runs). Host
  transposes to [d, y, x].

  DMA work is spread across all three TRN2 queue paths (qSP, qActHW,
  qPool): input loads + shift DMAs alternate sync/scalar by block parity,
  compact writes go on gpsimd. i2 is loaded contiguously and re-strided
  into its zero-padded layout on-chip (the padded-interior DMA pattern had
  256B elements).
"""

import numpy as np

C = 256
H, W = 48, 64
ND = 21          # displacements per axis
D = ND * ND      # 441
SUB_H, SUB_W = H // 2, W // 2      # 24, 32
HP, WP = H + 40, W + 40            # padded full-res 88, 104
BAND_W = 52                        # padded sub-cols
BAND_ROWS = 24                     # window sub-rows per block
BAND_N = BAND_ROWS * BAND_W        # 1248
SPAN = BAND_W * (ND - 1) + ND      # 1061: shifted row needed per pixel
ROW_SPLITS = [(0, 9), (9, 18), (18, 24)]   # window-row ranges per PSUM bank
N_BLOCKS = SUB_H // 4              # 6

_CACHE = {}


def _build():
    import concourse.bacc as bacc
    import concourse.mybir as mybir
    from concourse.bass_types import AP, SBTensorHandle
    from concourse.tile import TileContext

    f32 = mybir.dt.float32
    f32r = mybir.dt.float32r
    bf16 = mybir.dt.bfloat16

    def alias_sbuf(nc, name, shape, dtype, offset, base_partition):
        # SBUF tensor view at a fixed byte offset and nonzero base partition.
        # Rebases the partition origin so diagonal APs (flat SBUF addressing
        # couples partition and byte strides) keep their offset inside one
        # partition row (walrus rejects partition-crossing offsets there).
        uname = nc._get_name(name, add_next_id=True)
        nc._tensor(uname, list(shape), dtype, type="SB")
        import functools, operator
        per_part = functools.reduce(operator.mul, shape[1:]) * mybir.dt.size(dtype)
        h = SBTensorHandle(
            uname,
            list(shape),
            dtype,
            base_partition=base_partition,
            manual_sbuf_range=(offset, offset + per_part),
            manual_base_name=name,
        )
        mloc = nc.lookup_mloc(h)
        mloc.allocated = True
        mloc.addr = offset
        mloc.base = base_partition
        return h

    nc = bacc.Bacc("TRN2", target_bir_lowering=False, debug=False)
    i1_t = nc.dram_tensor("i1", [C, H, W], f32, kind="ExternalInput")
    i2_t = nc.dram_tensor("i2", [C, H, W], f32, kind="ExternalInput")
    od_t = nc.dram_tensor("od", [H, W, D], f32, kind="ExternalOutput")

    NBUF = 3
    band_full = []
    band_alias = []
    for i in range(NBUF):
        h = nc.alloc_sbuf_tensor(f"bandf{i}", [128, BAND_N], bf16)
        addr = nc.lookup_mloc(h).addr
        band_full.append(h)
        band_alias.append(
            [
                alias_sbuf(nc, f"band{i}ry{ry}", [32, BAND_N], bf16, addr, 32 * ry)
                for ry in range(4)
            ]
        )

    from bass_rust import add_dep_helper

    last_shift = [[] for _ in range(NBUF)]

    with TileContext(nc) as tc:
        with (
            tc.tile_pool(name="inp", bufs=1) as inp_pool,
            tc.tile_pool(name="work", bufs=3) as work_pool,
            tc.tile_pool(name="ps", bufs=2, space="PSUM") as ps_pool,
        ):
            i1_sb = [
                inp_pool.tile([128, H * W], f32, name=f"i1k{k}", tag=f"i1k{k}") for k in range(2)
            ]
            i2_st = [
                inp_pool.tile([128, H * W], f32, name=f"i2st{k}", tag=f"i2st{k}") for k in range(2)
            ]
            i2_sb = [
                inp_pool.tile([128, HP * WP], bf16, name=f"i2k{k}", tag=f"i2k{k}") for k in range(2)
            ]
            i1s_sb = [
                [
                    inp_pool.tile(
                        [128, SUB_H * SUB_W], bf16, name=f"i1s{k}{s}", tag=f"i1s{k}{s}"
                    )
                    for s in range(4)
                ]
                for k in range(2)
            ]
            i1v = [t[:].rearrange("c (h w) -> c h w", h=H) for t in i1_sb]
            i2v = [t[:].rearrange("c (h w) -> c h w", h=HP) for t in i2_sb]

            for k in range(2):
                cs = slice(128 * k, 128 * (k + 1))
                q = nc.sync if k == 0 else nc.scalar
                q.dma_start(out=i1_sb[k][:], in_=i1_t.ap()[cs])
                q.dma_start(out=i2_st[k][:], in_=i2_t.ap()[cs])
                v = i2v[k]
                # zero the pad ring (gpsimd; disjoint from the interior copy)
                nc.gpsimd.memset(v[:, 0:20, :], 0.0)
                nc.gpsimd.memset(v[:, 68:HP, :], 0.0)
                nc.gpsimd.memset(v[:, 20:68, 0:20], 0.0)
                nc.gpsimd.memset(v[:, 20:68, 84:WP], 0.0)
                # re-stride the contiguous staging load into the padded frame
                nc.vector.tensor_copy(
                    v[:, 20:68, 20:84],
                    i2_st[k][:].rearrange("c (h w) -> c h w", h=H),
                )
                # de-interleave i1 into the 4 polyphase sub-images:
                # stationary matmul operands need a single-stride free dim.
                # k0 on vector (needed first), k1 on gpsimd (overlaps).
                for s in range(4):
                    sy, sx = s >> 1, s & 1
                    eng = nc.vector if k == 0 else nc.gpsimd
                    eng.tensor_copy(
                        i1s_sb[k][s][:].rearrange(
                            "c (py px) -> c py px", py=SUB_H
                        ),
                        i1v[k][:, sy : sy + 2 * SUB_H - 1 : 2, sx::2],
                    )

            inv_c = 1.0 / C
            for s in range(4):
                sy, sx = s >> 1, s & 1
                for yb in range(N_BLOCKS):
                    Y = 4 * yb
                    t = s * N_BLOCKS + yb
                    ps = ps_pool.tile([128, 1536], f32, name="ps")
                    for j, (r0, r1) in enumerate(ROW_SPLITS):
                        n = (r1 - r0) * BAND_W
                        for k in range(2):
                            lhs = i1s_sb[k][s][:, 32 * Y : 32 * Y + 128]
                            rh = i2v[k][
                                :,
                                2 * (Y + r0) + sy : 2 * (Y + r1 - 1) + sy + 1 : 2,
                                sx::2,
                            ]
                            nc.tensor.matmul(
                                ps[:, 512 * j : 512 * j + n],
                                lhsT=lhs,
                                rhs=rh,
                                start=(k == 0),
                                stop=(k == 1),
                            )
                    bi = t % NBUF
                    band = band_full[bi].ap()
                    # A: evict PSUM -> bf16 band (cast copy, vector)
                    copies = [
                        nc.vector.tensor_copy(band[:, 0:468], ps[:, 0:468]),
                        nc.vector.tensor_copy(band[:, 468:936], ps[:, 512:980]),
                        nc.vector.tensor_copy(band[:, 936:1248], ps[:, 1024:1336]),
                    ]
                    # band lives outside the tile pools (aliases rebase
                    # partitions, which Tile can't track): explicit RAW
                    # (shift-after-copy) and WAR (copy-after-shift on reuse).
                    for c in copies:
                        for g in last_shift[bi]:
                            add_dep_helper(c.ins, g.ins, reason="band WAR")
                    # B: per-partition diagonal shift, one DMA per ry-group,
                    # whole-pixel-row elements (1061 x bf16 = 2122B each).
                    band2 = work_pool.tile([128, ND * BAND_W], bf16, name="b2", tag="b2")
                    q = nc.sync if (t & 1) == 0 else nc.scalar
                    shifts = []
                    for ry in range(4):
                        rd = AP(
                            band_alias[bi][ry],
                            ry * BAND_W,
                            [[BAND_N + 1, 32], [1, SPAN]],
                        )
                        g = q.dma_start(out=band2[32 * ry : 32 * ry + 32, 0:SPAN], in_=rd)
                        for c in copies:
                            add_dep_helper(g.ins, c.ins, reason="band RAW")
                        shifts.append(g)
                    last_shift[bi] = shifts
                    # C: regular re-stride 52->21 + 1/C scale, bf16 -> fp32
                    compact = work_pool.tile([128, D], f32, name="cp", tag="cp")
                    b2v = band2[:].rearrange("p (dy u) -> p dy u", dy=ND)
                    cpv = compact[:].rearrange("p (dy dx) -> p dy dx", dy=ND)
                    nc.vector.tensor_scalar_mul(cpv, b2v[:, :, 0:ND], inv_c)
                    # D: contiguous compact -> HBM [y, x, d] (gpsimd SWDGE
                    # queue; read side is fully regular 1764B rows).
                    base = ((8 * yb + sy) * W + sx) * D
                    wr = AP(
                        od_t.ap().tensor,
                        base,
                        [[2 * W * D, 4], [2 * D, 32], [1, D]],
                    )
                    nc.gpsimd.dma_start(out=wr, in_=compact[:])

    nc.compile()
    return nc


def _get_program():
    if "nc" not in _CACHE:
        _CACHE["nc"] = _build()
    return _CACHE["nc"]


def kernel(input1: np.ndarray, input2: np.ndarray) -> np.ndarray:
    from concourse import bass_utils

    nc = _get_program()
    input1 = np.ascontiguousarray(input1, dtype=np.float32)
    input2 = np.ascontiguousarray(input2, dtype=np.float32)
    B = input1.shape[0]
    in_maps = [{"i1": input1[b], "i2": input2[b]} for b in range(B)]
    res = bass_utils.run_bass_kernel_spmd(nc, in_maps, core_ids=list(range(B)))
    out = np.stack([r["od"] for r in res.results])  # [B, H, W, D]
    return np.ascontiguousarray(out.transpose(0, 3, 1, 2))  # [B, D, H, W]


# revision 8
# speedup vs baseline: 2.0652x; 1.0574x over previous
"""FlowNet correlation kernel for Trainium2 (8 NeuronCores, batch-parallel).

Problem: out[b, d, y, x] = (1/C) * sum_c i1[b,c,y,x] * pad(i2)[b,c,y+dy,x+dx]
  B=8, C=256, H=48, W=64, pad=20, displacements dy,dx in {-20..20 step 2}
  (21x21 = 441), output [8, 441, 48, 64] fp32.

Strategy (per core, one batch element):
  Displacement stride 2 => 4 polyphase subproblems (sy, sx), each a dense
  +-10 correlation on a 24x32 quarter image. Per subproblem, blocks of 4
  sub-rows (M = 4*32 = 128 pixels): all-pairs band via bf16 matmuls
  (1 cycle/row, 4x fp32 rate; inputs convert during the on-chip
  restride/deinterleave copies), band = 24 win-rows x 52
  win-cols = 1248, accumulated over two 128-channel k-tiles.

  Extraction avoids tiny-element DMAs (the old 84B diagonal gathers ran at
  ~6ns/elem contended): PSUM -> bf16 band (vector, cast), then ONE
  per-partition diagonal *shift* DMA per ry-group (elements are whole
  1061-elem rows, 2.1KB), then a regular vector re-stride 52->21 with the
  1/C scale into a compact [pixel, 441] fp32 tile, then a fully regular
  contiguous write DMA to HBM ([y, x, d] layout, 1764B runs). Host
  transposes to [d, y, x].

  DMA work is spread across all three TRN2 queue paths (qSP, qActHW,
  qPool): input loads + shift DMAs alternate sync/scalar by block parity,
  compact writes go on gpsimd. i2 is loaded contiguously and re-strided
  into its zero-padded layout on-chip (the padded-interior DMA pattern had
  256B elements).
"""

import numpy as np

C = 256
H, W = 48, 64
ND = 21          # displacements per axis
D = ND * ND      # 441
SUB_H, SUB_W = H // 2, W // 2      # 24, 32
HP, WP = H + 40, W + 40            # padded full-res 88, 104
BAND_W = 52                        # padded sub-cols
BAND_ROWS = 24                     # window sub-rows per block
BAND_N = BAND_ROWS * BAND_W        # 1248
SPAN = BAND_W * (ND - 1) + ND      # 1061: shifted row needed per pixel
ROW_SPLITS = [(0, 9), (9, 18), (18, 24)]   # window-row ranges per PSUM bank
N_BLOCKS = SUB_H // 4              # 6

_CACHE = {}


def _build():
    import concourse.bacc as bacc
    import concourse.mybir as mybir
    from concourse.bass_types import AP, SBTensorHandle
    from concourse.tile import TileContext

    f32 = mybir.dt.float32
    f32r = mybir.dt.float32r
    bf16 = mybir.dt.bfloat16

    def alias_sbuf(nc, name, shape, dtype, offset, base_partition):
        # SBUF tensor view at a fixed byte offset and nonzero base partition.
        # Rebases the partition origin so diagonal APs (flat SBUF addressing
        # couples partition and byte strides) keep their offset inside one
        # partition row (walrus rejects partition-crossing offsets there).
        uname = nc._get_name(name, add_next_id=True)
        nc._tensor(uname, list(shape), dtype, type="SB")
        import functools, operator
        per_part = functools.reduce(operator.mul, shape[1:]) * mybir.dt.size(dtype)
        h = SBTensorHandle(
            uname,
            list(shape),
            dtype,
            base_partition=base_partition,
            manual_sbuf_range=(offset, offset + per_part),
            manual_base_name=name,
        )
        mloc = nc.lookup_mloc(h)
        mloc.allocated = True
        mloc.addr = offset
        mloc.base = base_partition
        return h

    nc = bacc.Bacc("TRN2", target_bir_lowering=False, debug=False)
    i1_t = nc.dram_tensor("i1", [C, H, W], f32, kind="ExternalInput")
    i2_t = nc.dram_tensor("i2", [C, H, W], f32, kind="ExternalInput")
    od_t = nc.dram_tensor("od", [H, W, D], f32, kind="ExternalOutput")

    NBUF = 3
    band_full = []
    band_alias = []
    for i in range(NBUF):
        h = nc.alloc_sbuf_tensor(f"bandf{i}", [128, BAND_N], bf16)
        addr = nc.lookup_mloc(h).addr
        band_full.append(h)
        band_alias.append(
            [
                alias_sbuf(nc, f"band{i}ry{ry}", [32, BAND_N], bf16, addr, 32 * ry)
                for ry in range(4)
            ]
        )

    from bass_rust import add_dep_helper

    last_shift = [[] for _ in range(NBUF)]

    with TileContext(nc) as tc:
        with (
            tc.tile_pool(name="inp", bufs=1) as inp_pool,
            tc.tile_pool(name="work", bufs=3) as work_pool,
            tc.tile_pool(name="ps", bufs=2, space="PSUM") as ps_pool,
        ):
            i1_sb = [
                inp_pool.tile([128, H * W], f32, name=f"i1k{k}", tag=f"i1k{k}") for k in range(2)
            ]
            i2_st = [
                inp_pool.tile([128, H * W], f32, name=f"i2st{k}", tag=f"i2st{k}") for k in range(2)
            ]
            i2_sb = [
                inp_pool.tile([128, HP * WP], bf16, name=f"i2k{k}", tag=f"i2k{k}") for k in range(2)
            ]
            i1s_sb = [
                [
                    inp_pool.tile(
                        [128, SUB_H * SUB_W], bf16, name=f"i1s{k}{s}", tag=f"i1s{k}{s}"
                    )
                    for s in range(4)
                ]
                for k in range(2)
            ]
            i1v = [t[:].rearrange("c (h w) -> c h w", h=H) for t in i1_sb]
            i2v = [t[:].rearrange("c (h w) -> c h w", h=HP) for t in i2_sb]

            for k in range(2):
                cs = slice(128 * k, 128 * (k + 1))
                q = nc.sync if k == 0 else nc.scalar
                q.dma_start(out=i1_sb[k][:], in_=i1_t.ap()[cs])
                q.dma_start(out=i2_st[k][:], in_=i2_t.ap()[cs])
                v = i2v[k]
                # zero the pad ring (gpsimd; disjoint from the interior copy)
                nc.gpsimd.memset(v[:, 0:20, :], 0.0)
                nc.gpsimd.memset(v[:, 68:HP, :], 0.0)
                nc.gpsimd.memset(v[:, 20:68, 0:20], 0.0)
                nc.gpsimd.memset(v[:, 20:68, 84:WP], 0.0)
                # re-stride the contiguous staging load into the padded frame
                nc.vector.tensor_copy(
                    v[:, 20:68, 20:84],
                    i2_st[k][:].rearrange("c (h w) -> c h w", h=H),
                )
                # de-interleave i1 into the 4 polyphase sub-images:
                # stationary matmul operands need a single-stride free dim.
                # all on vector: it is idle in the prologue and gpsimd
                # casts run ~6x slower.
                for s in range(4):
                    sy, sx = s >> 1, s & 1
                    eng = nc.vector
                    eng.tensor_copy(
                        i1s_sb[k][s][:].rearrange(
                            "c (py px) -> c py px", py=SUB_H
                        ),
                        i1v[k][:, sy : sy + 2 * SUB_H - 1 : 2, sx::2],
                    )

            inv_c = 1.0 / C
            LAG = 2
            pend = []

            def flush(item):
                band2, base = item
                # C: regular re-stride 52->21 + 1/C scale, bf16 -> fp32
                compact = work_pool.tile([128, D], f32, name="cp", tag="cp")
                b2v = band2[:].rearrange("p (dy u) -> p dy u", dy=ND)
                cpv = compact[:].rearrange("p (dy dx) -> p dy dx", dy=ND)
                nc.vector.tensor_scalar_mul(cpv, b2v[:, :, 0:ND], inv_c)
                # D: contiguous compact -> HBM [y, x, d] (gpsimd SWDGE
                # queue; read side is fully regular 1764B rows).
                wr = AP(
                    od_t.ap().tensor,
                    base,
                    [[2 * W * D, 4], [2 * D, 32], [1, D]],
                )
                nc.gpsimd.dma_start(out=wr, in_=compact[:])

            for s in range(4):
                sy, sx = s >> 1, s & 1
                for yb in range(N_BLOCKS):
                    Y = 4 * yb
                    t = s * N_BLOCKS + yb
                    ps = ps_pool.tile([128, 1536], f32, name="ps")
                    for j, (r0, r1) in enumerate(ROW_SPLITS):
                        n = (r1 - r0) * BAND_W
                        for k in range(2):
                            lhs = i1s_sb[k][s][:, 32 * Y : 32 * Y + 128]
                            rh = i2v[k][
                                :,
                                2 * (Y + r0) + sy : 2 * (Y + r1 - 1) + sy + 1 : 2,
                                sx::2,
                            ]
                            nc.tensor.matmul(
                                ps[:, 512 * j : 512 * j + n],
                                lhsT=lhs,
                                rhs=rh,
                                start=(k == 0),
                                stop=(k == 1),
                            )
                    bi = t % NBUF
                    band = band_full[bi].ap()
                    # A: evict PSUM -> bf16 band (cast copies; the 312-col
                    # slice goes to scalar to offload vector)
                    copies = [
                        nc.vector.tensor_copy(band[:, 0:468], ps[:, 0:468]),
                        nc.vector.tensor_copy(band[:, 468:936], ps[:, 512:980]),
                        nc.scalar.copy(band[:, 936:1248], ps[:, 1024:1336]),
                    ]
                    # band lives outside the tile pools (aliases rebase
                    # partitions, which Tile can't track): explicit RAW
                    # (shift-after-copy) and WAR (copy-after-shift on reuse).
                    for c in copies:
                        for g in last_shift[bi]:
                            add_dep_helper(c.ins, g.ins, reason="band WAR")
                    # B: per-partition diagonal shift, one DMA per ry-group,
                    # whole-pixel-row elements (1061 x bf16 = 2122B each).
                    band2 = work_pool.tile([128, ND * BAND_W], bf16, name="b2", tag="b2")
                    q = nc.sync if (t & 1) == 0 else nc.scalar
                    shifts = []
                    for ry in range(4):
                        rd = AP(
                            band_alias[bi][ry],
                            ry * BAND_W,
                            [[BAND_N + 1, 32], [1, SPAN]],
                        )
                        g = q.dma_start(out=band2[32 * ry : 32 * ry + 32, 0:SPAN], in_=rd)
                        for c in copies:
                            add_dep_helper(g.ins, c.ins, reason="band RAW")
                        shifts.append(g)
                    last_shift[bi] = shifts
                    pend.append((band2, ((8 * yb + sy) * W + sx) * D))
                    # C+D lagged 2 blocks: the in-order vector stream never
                    # stalls on an in-flight shift DMA.
                    if len(pend) > LAG:
                        flush(pend.pop(0))
            while pend:
                flush(pend.pop(0))

    nc.compile()
    return nc


def _get_program():
    if "nc" not in _CACHE:
        _CACHE["nc"] = _build()
    return _CACHE["nc"]


def kernel(input1: np.ndarray, input2: np.ndarray) -> np.ndarray:
    from concourse import bass_utils

    nc = _get_program()
    input1 = np.ascontiguousarray(input1, dtype=np.float32)
    input2 = np.ascontiguousarray(input2, dtype=np.float32)
    B = input1.shape[0]
    in_maps = [{"i1": input1[b], "i2": input2[b]} for b in range(B)]
    res = bass_utils.run_bass_kernel_spmd(nc, in_maps, core_ids=list(range(B)))
    out = np.stack([r["od"] for r in res.results])  # [B, H, W, D]
    return np.ascontiguousarray(out.transpose(0, 3, 1, 2))  # [B, D, H, W]


# revision 13
# speedup vs baseline: 3.5471x; 1.7175x over previous
"""FlowNet correlation kernel for Trainium2 (8 NeuronCores, batch-parallel).

Problem: out[b, d, y, x] = (1/C) * sum_c i1[b,c,y,x] * pad(i2)[b,c,y+dy,x+dx]
  B=8, C=256, H=48, W=64, pad=20, displacements dy,dx in {-20..20 step 2}
  (21x21 = 441), output [8, 441, 48, 64] fp32.

Strategy (per core, one batch element):
  Displacement stride 2 => 4 polyphase subproblems (sy, sx), each a dense
  +-10 correlation on a 24x32 quarter image. Per subproblem, blocks of 4
  sub-rows (M = 4*32 = 128 pixels): all-pairs band via bf16 matmuls
  (1 cycle/row, 4x fp32 rate; inputs convert during the on-chip
  restride/deinterleave copies), band = 24 win-rows x 52
  win-cols = 1248, accumulated over two 128-channel k-tiles.

  Extraction avoids tiny-element DMAs (the old 84B diagonal gathers ran at
  ~6ns/elem contended): PSUM -> bf16 band (vector, cast), then ONE
  per-partition diagonal *shift* DMA per ry-group (elements are whole
  1061-elem rows, 2.1KB), then a regular vector re-stride 52->21 with the
  1/C scale into a compact [pixel, 441] fp32 tile, then a fully regular
  contiguous write DMA to HBM ([y, x, d] layout, 1764B runs). Host
  transposes to [d, y, x].

  DMA work is spread across all three TRN2 queue paths (qSP, qActHW,
  qPool): input loads + shift DMAs alternate sync/scalar by block parity,
  compact writes go on gpsimd. i2 is loaded contiguously and re-strided
  into its zero-padded layout on-chip (the padded-interior DMA pattern had
  256B elements).
"""

import numpy as np

C = 256
H, W = 48, 64
ND = 21          # displacements per axis
D = ND * ND      # 441
SUB_H, SUB_W = H // 2, W // 2      # 24, 32
HP, WP = H + 40, W + 40            # padded full-res 88, 104
BAND_W = 52                        # padded sub-cols
BAND_ROWS = 24                     # window sub-rows per block
BAND_N = BAND_ROWS * BAND_W        # 1248
SPAN = BAND_W * (ND - 1) + ND      # 1061: shifted row needed per pixel
ROW_SPLITS = [(0, 9), (9, 18), (18, 24)]   # window-row ranges per PSUM bank
N_BLOCKS = SUB_H // 4              # 6

_CACHE = {}


def _build():
    import concourse.bacc as bacc
    import concourse.mybir as mybir
    from concourse.bass_types import AP, SBTensorHandle
    from concourse.tile import TileContext

    f32 = mybir.dt.float32
    f32r = mybir.dt.float32r
    bf16 = mybir.dt.bfloat16

    def alias_sbuf(nc, name, shape, dtype, offset, base_partition):
        # SBUF tensor view at a fixed byte offset and nonzero base partition.
        # Rebases the partition origin so diagonal APs (flat SBUF addressing
        # couples partition and byte strides) keep their offset inside one
        # partition row (walrus rejects partition-crossing offsets there).
        uname = nc._get_name(name, add_next_id=True)
        nc._tensor(uname, list(shape), dtype, type="SB")
        import functools, operator
        per_part = functools.reduce(operator.mul, shape[1:]) * mybir.dt.size(dtype)
        h = SBTensorHandle(
            uname,
            list(shape),
            dtype,
            base_partition=base_partition,
            manual_sbuf_range=(offset, offset + per_part),
            manual_base_name=name,
        )
        mloc = nc.lookup_mloc(h)
        mloc.allocated = True
        mloc.addr = offset
        mloc.base = base_partition
        return h

    nc = bacc.Bacc("TRN2", target_bir_lowering=False, debug=False)
    i1_t = nc.dram_tensor("i1", [C, H, W], f32, kind="ExternalInput")
    i2_t = nc.dram_tensor("i2", [C, H, W], f32, kind="ExternalInput")
    od_t = nc.dram_tensor("od", [H, W, D], f32, kind="ExternalOutput")

    NBUF = 2  # pair-buffers: each holds two consecutive blocks side by side
    band_full = []
    band_alias = []
    for i in range(NBUF):
        h = nc.alloc_sbuf_tensor(f"bandf{i}", [128, 2 * BAND_N], bf16)
        addr = nc.lookup_mloc(h).addr
        band_full.append(h)
        band_alias.append(
            [
                alias_sbuf(nc, f"band{i}ry{ry}", [32, 2 * BAND_N], bf16, addr, 32 * ry)
                for ry in range(4)
            ]
        )

    from bass_rust import add_dep_helper

    last_c = [[] for _ in range(NBUF)]

    with TileContext(nc) as tc:
        with (
            tc.tile_pool(name="inp", bufs=1) as inp_pool,
            tc.tile_pool(name="work", bufs=3) as work_pool,
            tc.tile_pool(name="ps", bufs=2, space="PSUM") as ps_pool,
        ):
            i1_sb = [
                inp_pool.tile([128, H * W], f32, name=f"i1k{k}", tag=f"i1k{k}") for k in range(2)
            ]
            i2_st = [
                inp_pool.tile([128, H * W], f32, name=f"i2st{k}", tag=f"i2st{k}") for k in range(2)
            ]
            i2_sb = [
                inp_pool.tile([128, HP * WP], bf16, name=f"i2k{k}", tag=f"i2k{k}") for k in range(2)
            ]
            i1s_sb = [
                [
                    inp_pool.tile(
                        [128, SUB_H * SUB_W], bf16, name=f"i1s{k}{s}", tag=f"i1s{k}{s}"
                    )
                    for s in range(4)
                ]
                for k in range(2)
            ]
            i1v = [t[:].rearrange("c (h w) -> c h w", h=H) for t in i1_sb]
            i2v = [t[:].rearrange("c (h w) -> c h w", h=HP) for t in i2_sb]

            for k in range(2):
                cs = slice(128 * k, 128 * (k + 1))
                q = nc.sync if k == 0 else nc.scalar
                q.dma_start(out=i1_sb[k][:], in_=i1_t.ap()[cs])
                q.dma_start(out=i2_st[k][:], in_=i2_t.ap()[cs])
                v = i2v[k]
                # zero the pad ring (gpsimd; disjoint from the interior copy)
                nc.gpsimd.memset(v[:, 0:20, :], 0.0)
                nc.gpsimd.memset(v[:, 68:HP, :], 0.0)
                nc.gpsimd.memset(v[:, 20:68, 0:20], 0.0)
                nc.gpsimd.memset(v[:, 20:68, 84:WP], 0.0)
                # re-stride the contiguous staging load into the padded frame
                nc.vector.tensor_copy(
                    v[:, 20:68, 20:84],
                    i2_st[k][:].rearrange("c (h w) -> c h w", h=H),
                )
                # de-interleave i1 into the 4 polyphase sub-images:
                # stationary matmul operands need a single-stride free dim.
                # all on vector: it is idle in the prologue and gpsimd
                # casts run ~6x slower.
                for s in range(4):
                    sy, sx = s >> 1, s & 1
                    eng = nc.vector
                    eng.tensor_copy(
                        i1s_sb[k][s][:].rearrange(
                            "c (py px) -> c py px", py=SUB_H
                        ),
                        i1v[k][:, sy : sy + 2 * SUB_H - 1 : 2, sx::2],
                    )

            inv_c = 1.0 / C
            LAG = 1  # in pairs
            pend = []
            pair_copies = []
            NB2 = ND * BAND_W  # 1092

            def flush(item):
                band2, base, fbi = item
                # C: re-stride 52->21 + 1/C scale, bf16 -> fp32, per half
                compact = work_pool.tile([128, 2 * D], f32, name="cp", tag="cp")
                muls = []
                for h2 in range(2):
                    b2v = band2[:, h2 * NB2 : (h2 + 1) * NB2].rearrange(
                        "p (dy u) -> p dy u", dy=ND
                    )
                    cpv = compact[:, h2 * D : (h2 + 1) * D].rearrange(
                        "p (dy dx) -> p dy dx", dy=ND
                    )
                    muls.append(nc.vector.tensor_scalar_mul(cpv, b2v[:, :, 0:ND], inv_c))
                # C starting proves the shift DMAs' transfers landed (Tile
                # adds their completion-sem waits), so the band-buffer WAR
                # for pair fbi+NBUF chains through these.
                last_c[fbi] = muls
                # D: contiguous writes, one per block half (partition-first
                # SBUF read; a pair-major read AP lowers incorrectly).
                for h2 in range(2):
                    wr = AP(
                        od_t.ap().tensor,
                        base + h2 * 8 * W * D,
                        [[2 * W * D, 4], [2 * D, 32], [1, D]],
                    )
                    nc.gpsimd.dma_start(out=wr, in_=compact[:, h2 * D : (h2 + 1) * D])

            for s in range(4):
                sy, sx = s >> 1, s & 1
                for yb in range(N_BLOCKS):
                    Y = 4 * yb
                    t = s * N_BLOCKS + yb
                    ps = ps_pool.tile([128, 1536], f32, name="ps")
                    for j, (r0, r1) in enumerate(ROW_SPLITS):
                        n = (r1 - r0) * BAND_W
                        for k in range(2):
                            lhs = i1s_sb[k][s][:, 32 * Y : 32 * Y + 128]
                            rh = i2v[k][
                                :,
                                2 * (Y + r0) + sy : 2 * (Y + r1 - 1) + sy + 1 : 2,
                                sx::2,
                            ]
                            nc.tensor.matmul(
                                ps[:, 512 * j : 512 * j + n],
                                lhsT=lhs,
                                rhs=rh,
                                start=(k == 0),
                                stop=(k == 1),
                            )
                    bi = (t // 2) % NBUF
                    h = t & 1
                    off = h * BAND_N
                    band = band_full[bi].ap()
                    # A: evict PSUM -> bf16 band half (cast copies; the
                    # 312-col slice goes to scalar to offload vector)
                    copies = [
                        nc.vector.tensor_copy(band[:, off : off + 468], ps[:, 0:468]),
                        nc.vector.tensor_copy(band[:, off + 468 : off + 936], ps[:, 512:980]),
                        nc.scalar.copy(band[:, off + 936 : off + 1248], ps[:, 1024:1336]),
                    ]
                    # band lives outside the tile pools (aliases rebase
                    # partitions, which Tile can't track): explicit RAW
                    # (shift-after-copy); WAR on buffer reuse chains through
                    # the C stage (which waits the shift DMA completion sems
                    # -- a dep on the async DMA instruction itself would only
                    # order against its *issue*, racing the in-flight read).
                    for c in copies:
                        for m in last_c[bi]:
                            add_dep_helper(c.ins, m.ins, reason="band WAR")
                    pair_copies += copies
                    if h == 0:
                        continue
                    # B: per-partition diagonal shift, merged over the pair:
                    # one DMA per ry-group, 64 elements of 2122B (keeps more
                    # DMA engines busy per transfer), split across queues.
                    band2 = work_pool.tile([128, 2 * ND * BAND_W], bf16, name="b2", tag="b2")
                    shifts = []
                    for ry in range(4):
                        rd = AP(
                            band_alias[bi][ry],
                            ry * BAND_W,
                            [[2 * BAND_N + 1, 32], [BAND_N, 2], [1, SPAN]],
                        )
                        q = nc.sync if ry < 2 else nc.scalar
                        ov = band2[32 * ry : 32 * ry + 32, :].rearrange(
                            "p (t2 u) -> p t2 u", t2=2
                        )[:, :, 0:SPAN]
                        g = q.dma_start(out=ov, in_=rd)
                        for c in pair_copies:
                            add_dep_helper(g.ins, c.ins, reason="band RAW")
                        shifts.append(g)
                    pair_copies = []
                    pend.append((band2, ((8 * (yb - 1) + sy) * W + sx) * D, bi))
                    # C+D lagged one pair: the in-order vector stream never
                    # stalls on an in-flight shift DMA.
                    if len(pend) > LAG:
                        flush(pend.pop(0))
            while pend:
                flush(pend.pop(0))

    nc.compile()
    return nc


def _get_program():
    if "nc" not in _CACHE:
        _CACHE["nc"] = _build()
    return _CACHE["nc"]


def kernel(input1: np.ndarray, input2: np.ndarray) -> np.ndarray:
    from concourse import bass_utils

    nc = _get_program()
    input1 = np.ascontiguousarray(input1, dtype=np.float32)
    input2 = np.ascontiguousarray(input2, dtype=np.float32)
    B = input1.shape[0]
    in_maps = [{"i1": input1[b], "i2": input2[b]} for b in range(B)]
    res = bass_utils.run_bass_kernel_spmd(nc, in_maps, core_ids=list(range(B)))
    out = np.stack([r["od"] for r in res.results])  # [B, H, W, D]
    return np.ascontiguousarray(out.transpose(0, 3, 1, 2))  # [B, D, H, W]
